# revision 6
# baseline (speedup 1.0000x reference)
"""KAN layer (base linear + sin-basis spline) on 8 Trainium2 NeuronCores.

Math: out[r, o] = sum_i x[r,i] * bw[o,i] + sum_{i,g} sin(g * x[r,i]) * sw[o,i,g-1]

Strategy:
  - Fuse both einsums into one matmul with contraction K = 6*1024 = 6144:
    features = [x, sin(1x), ..., sin(5x)], weights = [bw.T, sw[..,0].T, ...].
  - Data-parallel over the 8192 (batch*seq) rows: 1024 rows per core,
    weights replicated (shipped pre-transposed + bf16 from host).
  - On device per 128-row block: DMA x -> PE-transpose to feature-major ->
    DVE range-reduce (g*x mod 2pi) -> ACT Sin -> bf16 feature bank ->
    PE matmul (K on partitions, fp32 PSUM accumulate) -> DVE copy -> DMA out.
  - ScalarE Sin is only valid on [-pi, pi]; we feed u = (g*x mod 2pi) with
    bias -pi, which yields sin(u - pi) = -sin(g*x), and absorb the sign by
    negating the spline weights on the host.
"""

import math
import sys

import numpy as np

sys.path.insert(0, "/opt/trn_rl_repo")

import ml_dtypes  # noqa: E402

B, S, IN, OUT, G = 4, 2048, 1024, 1024, 5
N_CORES = 8
ROWS = B * S                 # 8192
RPC = ROWS // N_CORES        # 1024 rows per core
NRB = RPC // 128             # 8 row-blocks per core
NF = G + 1                   # 6 feature groups (x, sin1..sin5)
NFB = IN // 128              # 8 feature blocks
KCHUNKS = NF * NFB           # 48 K-chunks of 128
TWO_PI = float(2.0 * math.pi)

_cache = {}


def _build_program():
    import concourse.tile as tile
    from concourse import bacc, mybir
    from concourse.masks import make_identity

    f32 = mybir.dt.float32
    bf16 = mybir.dt.bfloat16

    nc = bacc.Bacc("TRN2", target_bir_lowering=False, debug=False)
    x_ap = nc.dram_tensor("x", [RPC, IN], f32, kind="ExternalInput").ap()
    w_ap = nc.dram_tensor("w", [NF, 128, NFB * OUT], bf16, kind="ExternalInput").ap()
    out_ap = nc.dram_tensor("out", [RPC, OUT], f32, kind="ExternalOutput").ap()

    with tile.TileContext(nc) as tc:
        with (
            tc.tile_pool(name="const", bufs=1) as cpool,
            tc.tile_pool(name="wpool", bufs=1) as wpool,
            tc.tile_pool(name="xin", bufs=3) as xpool,
            tc.tile_pool(name="xt", bufs=2) as xtpool,
            tc.tile_pool(name="tmp", bufs=3) as tpool,
            tc.tile_pool(name="feat", bufs=2) as fpool,
            tc.tile_pool(name="ostage", bufs=3) as opool,
            tc.tile_pool(name="pst", bufs=4, space="PSUM") as pstpool,
            tc.tile_pool(name="pso", bufs=4, space="PSUM") as psopool,
        ):
            ident = cpool.tile([128, 128], f32, tag="ident")
            make_identity(nc, ident[:])

            # replicated weights, one tile per feature group (16KB/partition each)
            wt = []
            for g in range(NF):
                t = wpool.tile([128, NFB * OUT], bf16, tag=f"w{g}")
                nc.sync.dma_start(out=t[:], in_=w_ap[g])
                wt.append(t)

            for rb in range(NRB):
                xrow = xpool.tile([128, IN], f32)
                nc.sync.dma_start(out=xrow[:], in_=x_ap[rb * 128:(rb + 1) * 128, :])

                # transpose x row-block to feature-major via PE (4 per PSUM bank)
                xt32 = xtpool.tile([128, IN], f32)
                for half in range(2):
                    pst = pstpool.tile([128, 512], f32)
                    for j in range(4):
                        fb = half * 4 + j
                        nc.tensor.transpose(
                            pst[:, j * 128:(j + 1) * 128],
                            xrow[:, fb * 128:(fb + 1) * 128],
                            ident[:],
                        )
                    nc.vector.tensor_copy(
                        xt32[:, half * 512:(half + 1) * 512], pst[:]
                    )

                # feature bank [128, 6144] bf16: [xT, -sin(1 xT), ..., -sin(5 xT)]
                feat = fpool.tile([128, NF * IN], bf16)
                nc.vector.tensor_copy(feat[:, 0:IN], xt32[:])
                # range-reduce g*x to [-pi, pi] for the Sin LUT:
                # u = g*x/(2pi); n = rne(u) via the fp32 magic constant
                # (adding 1.5*2^23 forces round-to-nearest-integer);
                # sin(2pi*(u - n)) = sin(g*x)
                MAGIC = float(1.5 * 2 ** 23)
                for g in range(1, NF):
                    u = tpool.tile([128, IN], f32, tag="u")
                    nc.vector.tensor_scalar_mul(u[:], xt32[:], float(g) / TWO_PI)
                    n = tpool.tile([128, IN], f32, tag="n")
                    nc.vector.tensor_scalar(
                        n[:], u[:], MAGIC, MAGIC,
                        mybir.AluOpType.add, mybir.AluOpType.subtract,
                    )
                    nc.vector.tensor_sub(u[:], u[:], n[:])
                    nc.scalar.activation(
                        feat[:, g * IN:(g + 1) * IN], u[:],
                        mybir.ActivationFunctionType.Sin,
                        bias=0.0, scale=TWO_PI,
                    )

                for ob in range(2):
                    pso = psopool.tile([128, 512], f32)
                    for c in range(KCHUNKS):
                        g, fb = divmod(c, NFB)
                        nc.tensor.matmul(
                            pso[:],
                            lhsT=feat[:, c * 128:(c + 1) * 128],
                            rhs=wt[g][:, fb * OUT + ob * 512: fb * OUT + ob * 512 + 512],
                            start=(c == 0),
                            stop=(c == KCHUNKS - 1),
                        )
                    ostage = opool.tile([128, 512], f32)
                    nc.vector.tensor_copy(ostage[:], pso[:])
                    nc.sync.dma_start(
                        out=out_ap[rb * 128:(rb + 1) * 128, ob * 512:(ob + 1) * 512],
                        in_=ostage[:],
                    )
    nc.compile()
    return nc


def _host_weights(base_weight: np.ndarray, spline_weight: np.ndarray) -> np.ndarray:
    # W[g] laid out [partition(=feat%128), fb*OUT + o], k = g*1024 + fb*128 + p
    w = np.empty((NF, 128, NFB * OUT), dtype=np.float32)
    blocks = [base_weight.T]  # (IN, OUT)
    for g in range(1, NF):
        blocks.append(spline_weight[:, :, g - 1].T)
    for g, blk in enumerate(blocks):
        # (IN, OUT) -> (fb, 128, OUT) -> (128, fb, OUT) -> (128, fb*OUT)
        w[g] = blk.reshape(NFB, 128, OUT).transpose(1, 0, 2).reshape(128, NFB * OUT)
    return w.astype(ml_dtypes.bfloat16)


def kernel(x: np.ndarray, base_weight: np.ndarray, spline_weight: np.ndarray,
           _trace: bool = False):
    from concourse.bass_utils import run_bass_kernel_spmd

    if "nc" not in _cache:
        _cache["nc"] = _build_program()
    nc = _cache["nc"]

    w_host = _host_weights(np.asarray(base_weight, dtype=np.float32),
                           np.asarray(spline_weight, dtype=np.float32))
    x_flat = np.ascontiguousarray(np.asarray(x, dtype=np.float32).reshape(ROWS, IN))
    in_maps = [
        {"x": x_flat[i * RPC:(i + 1) * RPC], "w": w_host}
        for i in range(N_CORES)
    ]
    res = run_bass_kernel_spmd(nc, in_maps, core_ids=list(range(N_CORES)),
                               trace=_trace)
    out = np.concatenate([res.results[i]["out"] for i in range(N_CORES)], axis=0)
    if _trace:
        _cache["last_results"] = res
    return out.reshape(B, S, OUT).astype(np.float32)


# revision 9
# speedup vs baseline: 1.0828x; 1.0828x over previous
"""KAN layer (base linear + sin-basis spline) on 8 Trainium2 NeuronCores.

Math: out[r, o] = sum_i x[r,i] * bw[o,i] + sum_{i,g} sin(g * x[r,i]) * sw[o,i,g-1]

Strategy:
  - Fuse both einsums into one matmul with contraction K = 6*1024 = 6144:
    features = [x, sin(1x), ..., sin(5x)], weights = [bw.T, sw[..,0].T, ...].
  - Data-parallel over the 8192 (batch*seq) rows: 1024 rows per core,
    weights replicated (shipped pre-transposed + bf16 from host).
  - On device per 128-row block: DMA x -> PE-transpose to feature-major ->
    DVE range-reduce (g*x mod 2pi) -> ACT Sin -> bf16 feature bank ->
    PE matmul (K on partitions, fp32 PSUM accumulate) -> DVE copy -> DMA out.
  - ScalarE Sin is only valid on [-pi, pi]; we feed u = (g*x mod 2pi) with
    bias -pi, which yields sin(u - pi) = -sin(g*x), and absorb the sign by
    negating the spline weights on the host.
"""

import math
import sys

import numpy as np

sys.path.insert(0, "/opt/trn_rl_repo")

import ml_dtypes  # noqa: E402

B, S, IN, OUT, G = 4, 2048, 1024, 1024, 5
N_CORES = 8
ROWS = B * S                 # 8192
RPC = ROWS // N_CORES        # 1024 rows per core
NRB = RPC // 128             # 8 row-blocks per core
NF = G + 1                   # 6 feature groups (x, sin1..sin5)
NFB = IN // 128              # 8 feature blocks
KCHUNKS = NF * NFB           # 48 K-chunks of 128
TWO_PI = float(2.0 * math.pi)

_cache = {}


def _build_program():
    import concourse.tile as tile
    from concourse import bacc, mybir
    from concourse.masks import make_identity

    f32 = mybir.dt.float32
    bf16 = mybir.dt.bfloat16

    nc = bacc.Bacc("TRN2", target_bir_lowering=False, debug=False)
    x_ap = nc.dram_tensor("x", [RPC, IN], f32, kind="ExternalInput").ap()
    w_ap = nc.dram_tensor("w", [NF, 128, NFB * OUT], bf16, kind="ExternalInput").ap()
    out_ap = nc.dram_tensor("out", [RPC, OUT], f32, kind="ExternalOutput").ap()

    with tile.TileContext(nc) as tc:
        with (
            tc.tile_pool(name="const", bufs=1) as cpool,
            tc.tile_pool(name="wpool", bufs=1) as wpool,
            tc.tile_pool(name="xin", bufs=2) as xpool,
            tc.tile_pool(name="xt", bufs=2) as xtpool,
            tc.tile_pool(name="tmp", bufs=2) as tpool,
            tc.tile_pool(name="feat", bufs=4) as fpool,
            tc.tile_pool(name="ostage", bufs=3) as opool,
            tc.tile_pool(name="pst", bufs=2, space="PSUM") as pstpool,
            tc.tile_pool(name="pso", bufs=6, space="PSUM") as psopool,
        ):
            ident = cpool.tile([128, 128], f32, tag="ident")
            make_identity(nc, ident[:])

            # replicated weights, one tile per feature group (16KB/partition
            # each), issued on the ACT HWDGE ring so the 12.6MB load doesn't
            # head-of-line-block the x loads on the sync ring
            wt = []
            for g in range(NF):
                t = wpool.tile([128, NFB * OUT], bf16, tag=f"w{g}")
                nc.scalar.dma_start(out=t[:], in_=w_ap[g])
                wt.append(t)

            MAGIC = float(1.5 * 2 ** 23)

            def prep_feat(rb):
                """DMA x row-block, PE-transpose to feature-major, build the
                [xT, sin(1 xT), ..., sin(5 xT)] bf16 feature bank."""
                xrow = xpool.tile([128, IN], f32)
                nc.sync.dma_start(out=xrow[:], in_=x_ap[rb * 128:(rb + 1) * 128, :])
                xt32 = xtpool.tile([128, IN], f32)
                for half in range(2):
                    pst = pstpool.tile([128, 512], f32)
                    for j in range(4):
                        fb = half * 4 + j
                        nc.tensor.transpose(
                            pst[:, j * 128:(j + 1) * 128],
                            xrow[:, fb * 128:(fb + 1) * 128],
                            ident[:],
                        )
                    nc.vector.tensor_copy(
                        xt32[:, half * 512:(half + 1) * 512], pst[:]
                    )
                feat = fpool.tile([128, NF * IN], bf16)
                nc.vector.tensor_copy(feat[:, 0:IN], xt32[:])
                # range-reduce g*x to [-pi, pi] for the Sin LUT:
                # u = g*x/(2pi); n = rne(u) via the fp32 magic constant
                # (adding 1.5*2^23 forces round-to-nearest-integer);
                # sin(2pi*(u - n)) = sin(g*x)
                for g in range(1, NF):
                    u = tpool.tile([128, IN], f32, tag="u")
                    nc.vector.tensor_scalar_mul(u[:], xt32[:], float(g) / TWO_PI)
                    n = tpool.tile([128, IN], f32, tag="n")
                    nc.vector.tensor_scalar(
                        n[:], u[:], MAGIC, MAGIC,
                        mybir.AluOpType.add, mybir.AluOpType.subtract,
                    )
                    nc.vector.tensor_sub(u[:], u[:], n[:])
                    nc.scalar.activation(
                        feat[:, g * IN:(g + 1) * IN], u[:],
                        mybir.ActivationFunctionType.Sin,
                        bias=0.0, scale=TWO_PI,
                    )
                return feat

            def chunk_mm(pso, feat, c, ob, start, stop):
                g, fb = divmod(c, NFB)
                nc.tensor.matmul(
                    pso[:],
                    lhsT=feat[:, c * 128:(c + 1) * 128],
                    rhs=wt[g][:, fb * OUT + ob * 512: fb * OUT + ob * 512 + 512],
                    start=start, stop=stop,
                )

            def store(pso, rb, ob):
                ostage = opool.tile([128, 512], f32)
                nc.vector.tensor_copy(ostage[:], pso[:])
                nc.sync.dma_start(
                    out=out_ap[rb * 128:(rb + 1) * 128, ob * 512:(ob + 1) * 512],
                    in_=ostage[:],
                )

            # Phase 1 (weight-load shadow): 3 row-blocks, chunk-OUTER across
            # 6 open PSUM groups, so the in-order PE issues every matmul that
            # needs wt[g] only after ~g*10us of prior work -- no stall while
            # the weight tiles stream in.
            PRE = 3
            feats = [prep_feat(rb) for rb in range(PRE)]
            groups = [(rb, ob) for rb in range(PRE) for ob in range(2)]
            psos = []
            for i in range(len(groups)):
                pso_t = psopool.tile([128, 512], f32, tag="pso", name=f"pso_p1_{i}")
                psos.append(pso_t)
            for c in range(KCHUNKS):
                for i, (rb, ob) in enumerate(groups):
                    chunk_mm(psos[i], feats[rb], c, ob,
                             start=(c == 0), stop=(c == KCHUNKS - 1))
            for i, (rb, ob) in enumerate(groups):
                store(psos[i], rb, ob)

            # Phase 2: remaining row-blocks, chunk-inner (weights resident)
            for rb in range(PRE, NRB):
                feat = prep_feat(rb)
                for ob in range(2):
                    pso = psopool.tile([128, 512], f32, tag="pso")
                    for c in range(KCHUNKS):
                        chunk_mm(pso, feat, c, ob,
                                 start=(c == 0), stop=(c == KCHUNKS - 1))
                    store(pso, rb, ob)
    nc.compile()
    return nc


def _host_weights(base_weight: np.ndarray, spline_weight: np.ndarray) -> np.ndarray:
    # W[g] laid out [partition(=feat%128), fb*OUT + o], k = g*1024 + fb*128 + p
    w = np.empty((NF, 128, NFB * OUT), dtype=np.float32)
    blocks = [base_weight.T]  # (IN, OUT)
    for g in range(1, NF):
        blocks.append(spline_weight[:, :, g - 1].T)
    for g, blk in enumerate(blocks):
        # (IN, OUT) -> (fb, 128, OUT) -> (128, fb, OUT) -> (128, fb*OUT)
        w[g] = blk.reshape(NFB, 128, OUT).transpose(1, 0, 2).reshape(128, NFB * OUT)
    return w.astype(ml_dtypes.bfloat16)


def kernel(x: np.ndarray, base_weight: np.ndarray, spline_weight: np.ndarray,
           _trace: bool = False):
    from concourse.bass_utils import run_bass_kernel_spmd

    if "nc" not in _cache:
        _cache["nc"] = _build_program()
    nc = _cache["nc"]

    w_host = _host_weights(np.asarray(base_weight, dtype=np.float32),
                           np.asarray(spline_weight, dtype=np.float32))
    x_flat = np.ascontiguousarray(np.asarray(x, dtype=np.float32).reshape(ROWS, IN))
    in_maps = [
        {"x": x_flat[i * RPC:(i + 1) * RPC], "w": w_host}
        for i in range(N_CORES)
    ]
    res = run_bass_kernel_spmd(nc, in_maps, core_ids=list(range(N_CORES)),
                               trace=_trace)
    out = np.concatenate([res.results[i]["out"] for i in range(N_CORES)], axis=0)
    if _trace:
        _cache["last_results"] = res
    return out.reshape(B, S, OUT).astype(np.float32)


# revision 10
# speedup vs baseline: 1.1493x; 1.0614x over previous
"""KAN layer (base linear + sin-basis spline) on 8 Trainium2 NeuronCores.

Math: out[r, o] = sum_i x[r,i] * bw[o,i] + sum_{i,g} sin(g * x[r,i]) * sw[o,i,g-1]

Strategy:
  - Fuse both einsums into one matmul with contraction K = 6*1024 = 6144:
    features = [x, sin(1x), ..., sin(5x)], weights = [bw.T, sw[..,0].T, ...].
  - Data-parallel over the 8192 (batch*seq) rows: 1024 rows per core,
    weights replicated (shipped pre-transposed + bf16 from host).
  - On device per 128-row block: DMA x -> PE-transpose to feature-major ->
    DVE range-reduce (g*x mod 2pi) -> ACT Sin -> bf16 feature bank ->
    PE matmul (K on partitions, fp32 PSUM accumulate) -> DVE copy -> DMA out.
  - ScalarE Sin is only valid on [-pi, pi]; we feed u = (g*x mod 2pi) with
    bias -pi, which yields sin(u - pi) = -sin(g*x), and absorb the sign by
    negating the spline weights on the host.
"""

import math
import sys

import numpy as np

sys.path.insert(0, "/opt/trn_rl_repo")

import ml_dtypes  # noqa: E402

B, S, IN, OUT, G = 4, 2048, 1024, 1024, 5
N_CORES = 8
ROWS = B * S                 # 8192
RPC = ROWS // N_CORES        # 1024 rows per core
NRB = RPC // 128             # 8 row-blocks per core
NF = G + 1                   # 6 feature groups (x, sin1..sin5)
NFB = IN // 128              # 8 feature blocks
KCHUNKS = NF * NFB           # 48 K-chunks of 128
TWO_PI = float(2.0 * math.pi)

_cache = {}


def _build_program():
    import concourse.tile as tile
    from concourse import bacc, mybir
    from concourse.masks import make_identity

    f32 = mybir.dt.float32
    bf16 = mybir.dt.bfloat16

    nc = bacc.Bacc("TRN2", target_bir_lowering=False, debug=False)
    x_ap = nc.dram_tensor("x", [RPC, IN], f32, kind="ExternalInput").ap()
    w_ap = nc.dram_tensor("w", [NF, 128, NFB * OUT], bf16, kind="ExternalInput").ap()
    out_ap = nc.dram_tensor("out", [RPC, OUT], f32, kind="ExternalOutput").ap()

    with tile.TileContext(nc) as tc:
        with (
            tc.tile_pool(name="const", bufs=1) as cpool,
            tc.tile_pool(name="wpool", bufs=1) as wpool,
            tc.tile_pool(name="xin", bufs=2) as xpool,
            tc.tile_pool(name="xt", bufs=2) as xtpool,
            tc.tile_pool(name="tmp", bufs=2) as tpool,
            tc.tile_pool(name="feat", bufs=4) as fpool,
            tc.tile_pool(name="ostage", bufs=3) as opool,
            tc.tile_pool(name="pst", bufs=2, space="PSUM") as pstpool,
            tc.tile_pool(name="pso", bufs=6, space="PSUM") as psopool,
        ):
            ident = cpool.tile([128, 128], f32, tag="ident")
            make_identity(nc, ident[:])

            # replicated weights, one tile per feature group (16KB/partition
            # each), issued on the ACT HWDGE ring so the 12.6MB load doesn't
            # head-of-line-block the x loads on the sync ring
            wt = []
            for g in range(NF):
                t = wpool.tile([128, NFB * OUT], bf16, tag=f"w{g}")
                nc.scalar.dma_start(out=t[:], in_=w_ap[g])
                wt.append(t)

            MAGIC = float(1.5 * 2 ** 23)

            def prep_feat(rb):
                """DMA x row-block, PE-transpose to feature-major, build the
                [xT, sin(1 xT), ..., sin(5 xT)] bf16 feature bank."""
                xrow = xpool.tile([128, IN], f32)
                nc.sync.dma_start(out=xrow[:], in_=x_ap[rb * 128:(rb + 1) * 128, :])
                xt32 = xtpool.tile([128, IN], f32)
                for half in range(2):
                    pst = pstpool.tile([128, 512], f32)
                    for j in range(4):
                        fb = half * 4 + j
                        nc.tensor.transpose(
                            pst[:, j * 128:(j + 1) * 128],
                            xrow[:, fb * 128:(fb + 1) * 128],
                            ident[:],
                        )
                    nc.vector.tensor_copy(
                        xt32[:, half * 512:(half + 1) * 512], pst[:]
                    )
                feat = fpool.tile([128, NF * IN], bf16)
                nc.vector.tensor_copy(feat[:, 0:IN], xt32[:])
                # range-reduce g*x to [-pi, pi] for the Sin LUT:
                # u = g*x/(2pi); n = rne(u) via the fp32 magic constant
                # (adding 1.5*2^23 forces round-to-nearest-integer);
                # sin(2pi*(u - n)) = sin(g*x)
                for g in range(1, NF):
                    u = tpool.tile([128, IN], f32, tag="u")
                    nc.vector.tensor_scalar_mul(u[:], xt32[:], float(g) / TWO_PI)
                    n = tpool.tile([128, IN], f32, tag="n")
                    nc.vector.tensor_scalar(
                        n[:], u[:], MAGIC, MAGIC,
                        mybir.AluOpType.add, mybir.AluOpType.subtract,
                    )
                    nc.vector.tensor_sub(u[:], u[:], n[:])
                    nc.scalar.activation(
                        feat[:, g * IN:(g + 1) * IN], u[:],
                        mybir.ActivationFunctionType.Sin,
                        bias=0.0, scale=TWO_PI,
                    )
                return feat

            def chunk_mm(pso, feat, c, ob, start, stop):
                g, fb = divmod(c, NFB)
                nc.tensor.matmul(
                    pso[:],
                    lhsT=feat[:, c * 128:(c + 1) * 128],
                    rhs=wt[g][:, fb * OUT + ob * 512: fb * OUT + ob * 512 + 512],
                    start=start, stop=stop,
                )

            def store(pso, rb, ob):
                ostage = opool.tile([128, 512], f32)
                nc.vector.tensor_copy(ostage[:], pso[:])
                nc.sync.dma_start(
                    out=out_ap[rb * 128:(rb + 1) * 128, ob * 512:(ob + 1) * 512],
                    in_=ostage[:],
                )

            # Phase 1 (weight-load shadow): 2 row-blocks, chunk-OUTER across
            # 4 open PSUM groups, so the in-order PE issues every matmul that
            # needs wt[g] only after ~g*3.5us of prior work -- no stall while
            # the weight tiles stream in. rb2/rb3 feature prep is issued here
            # too, so their transposes run on PE before the matmul wall and
            # their DVE/ACT chains overlap the phase-1 matmuls.
            PRE = 2
            feats = {rb: prep_feat(rb) for rb in range(PRE + 2)}
            groups = [(rb, ob) for rb in range(PRE) for ob in range(2)]
            psos = []
            for i in range(len(groups)):
                pso_t = psopool.tile([128, 512], f32, tag="pso", name=f"pso_p1_{i}")
                psos.append(pso_t)
            for c in range(KCHUNKS):
                for i, (rb, ob) in enumerate(groups):
                    chunk_mm(psos[i], feats[rb], c, ob,
                             start=(c == 0), stop=(c == KCHUNKS - 1))

            # Phase 2: remaining row-blocks, chunk-inner (weights resident).
            # prep(rb+2) is issued BEFORE the stores so the next feature bank
            # is never queued behind PSUM-drain copies on the in-order DVE;
            # phase-1 stores drain while rb2 computes on the 2 spare banks.
            for rb in range(PRE, NRB):
                rb_psos = []
                for ob in range(2):
                    pso = psopool.tile([128, 512], f32, tag="pso")
                    for c in range(KCHUNKS):
                        chunk_mm(pso, feats[rb], c, ob,
                                 start=(c == 0), stop=(c == KCHUNKS - 1))
                    rb_psos.append(pso)
                if rb + 2 < NRB:
                    feats[rb + 2] = prep_feat(rb + 2)
                if rb == PRE:
                    for i, (prb, pob) in enumerate(groups):
                        store(psos[i], prb, pob)
                for ob in range(2):
                    store(rb_psos[ob], rb, ob)
    nc.compile()
    return nc


def _host_weights(base_weight: np.ndarray, spline_weight: np.ndarray) -> np.ndarray:
    # W[g] laid out [partition(=feat%128), fb*OUT + o], k = g*1024 + fb*128 + p
    w = np.empty((NF, 128, NFB * OUT), dtype=np.float32)
    blocks = [base_weight.T]  # (IN, OUT)
    for g in range(1, NF):
        blocks.append(spline_weight[:, :, g - 1].T)
    for g, blk in enumerate(blocks):
        # (IN, OUT) -> (fb, 128, OUT) -> (128, fb, OUT) -> (128, fb*OUT)
        w[g] = blk.reshape(NFB, 128, OUT).transpose(1, 0, 2).reshape(128, NFB * OUT)
    return w.astype(ml_dtypes.bfloat16)


def kernel(x: np.ndarray, base_weight: np.ndarray, spline_weight: np.ndarray,
           _trace: bool = False):
    from concourse.bass_utils import run_bass_kernel_spmd

    if "nc" not in _cache:
        _cache["nc"] = _build_program()
    nc = _cache["nc"]

    w_host = _host_weights(np.asarray(base_weight, dtype=np.float32),
                           np.asarray(spline_weight, dtype=np.float32))
    x_flat = np.ascontiguousarray(np.asarray(x, dtype=np.float32).reshape(ROWS, IN))
    in_maps = [
        {"x": x_flat[i * RPC:(i + 1) * RPC], "w": w_host}
        for i in range(N_CORES)
    ]
    res = run_bass_kernel_spmd(nc, in_maps, core_ids=list(range(N_CORES)),
                               trace=_trace)
    out = np.concatenate([res.results[i]["out"] for i in range(N_CORES)], axis=0)
    if _trace:
        _cache["last_results"] = res
    return out.reshape(B, S, OUT).astype(np.float32)
